# revision 11
# baseline (speedup 1.0000x reference)
"""Group-wise correlation cost volume (build_gwc_volume) on 8 trn2 cores.

volume[b,g,d,h,w] = sum_c ref[b,g,c,h,w] * tgt[b,g,c,h,w-d]  (0 where w<d)

Sharding: 16 (b,g) pairs across 8 cores, 2 pairs per core. Each pair is a
contiguous 64-channel slice of the inputs and a contiguous [D,H,W] slab of
the output.

Per (b,g,h) the volume rows are diagonals of the Gram matrix
G[w',w] = sum_c tgt[c,w'] * ref[c,w].  Only the band d = w - w' in [0,48)
is needed, so the Gram is computed as 8 column-piece matmuls (M=32,
stationary T[:, 32k:32k+32]), each with moving R[:, 32k : 32k+80) written
at free offset 0 of the PSUM tile.  Row p of the result then holds
G[p, 32*floor(p/32) + x] — the band sits in a fixed 80-wide window per
row (within-32 shear is resolved on the host).  The two (b,g) pairs sit
on PE row halves and the 4 column pieces on PE column groups, so all 8
matmuls per h run concurrently on the 128x128 array.

Band-tile bytes shipped: 80 + 47+33... per h-pair = 18432 els vs 12288
minimal. The diagonals are gathered on the host during unsharding.
"""

import sys

if "/opt/trn_rl_repo" not in sys.path:
    sys.path.insert(0, "/opt/trn_rl_repo")

import numpy as np

import concourse.bacc as bacc
import concourse.tile as tile
from concourse import mybir
from concourse.bass_utils import run_bass_kernel_spmd

F32 = mybir.dt.float32

B, C, H, W = 2, 512, 128, 256
G, CG, D = 8, 64, 48
N_CORES = 8
PAIRS = 2  # (b,g) pairs per core
HC = 8  # h rows per chunk
PW = 80  # piece window width (32 + 47 + 1)

# piece k covers w' in [32k, 32k+32); its moving window starts at
# BASE[k] = min(32k, W - PW) so every piece is a full 80 columns.
BASE = [min(32 * k, W - PW) for k in range(8)]

_cached = {}


def _build_module():
    nc = bacc.Bacc("TRN2", target_bir_lowering=False, debug=False, num_devices=N_CORES)
    ref = nc.dram_tensor("ref", [PAIRS, CG, H, W], F32, kind="ExternalInput")
    tgt = nc.dram_tensor("tgt", [PAIRS, CG, H, W], F32, kind="ExternalInput")
    # band tiles, layout [pair, w'-row, h, x]; tile0: w' in [0,128),
    # tile1 split by valid width: rows [128,192) x80, [192,224) x64, [224,256) x32
    out_bt = nc.dram_tensor(
        "out_bt", [PAIRS, 128, H, 2 * PW], F32, kind="ExternalOutput"
    )

    ref_p = ref.rearrange("pr c h w -> (pr c) h w")
    tgt_p = tgt.rearrange("pr c h w -> (pr c) h w")

    with tile.TileContext(nc) as tc:
        with (
            tc.tile_pool(name="ins", bufs=3) as ins,
            tc.tile_pool(name="stage", bufs=3) as stage_pool,
            tc.tile_pool(name="psum", bufs=4, space="PSUM") as psum,
        ):
            for ch in range(H // HC):
                h0 = ch * HC
                rt = ins.tile([128, HC, W], F32, tag="rt")
                tt = ins.tile([128, HC, W], F32, tag="tt")
                nc.sync.dma_start(rt[:], ref_p[:, h0 : h0 + HC, :])
                nc.gpsimd.dma_start(tt[:], tgt_p[:, h0 : h0 + HC, :])
                stages = []
                for pr in range(PAIRS):
                    st = stage_pool.tile(
                        [128, HC, 2 * PW], F32, tag=f"st{pr}", name=f"st{pr}_{ch}"
                    )
                    stages.append(st)
                for hl in range(HC):
                    for pr in range(PAIRS):
                        p0 = pr * CG
                        bank = psum.tile(
                            [128, 2 * PW], F32, tag=f"bk{pr}", name=f"bk{pr}_{ch}_{hl}"
                        )
                        for k in range(8):
                            c0 = PW * (k // 4)
                            m0 = 32 * (k % 4)
                            nc.tensor.matmul(
                                bank[m0 : m0 + 32, c0 : c0 + PW],
                                tt[p0 : p0 + CG, hl, 32 * k : 32 * k + 32],
                                rt[p0 : p0 + CG, hl, BASE[k] : BASE[k] + PW],
                                tile_position=(p0, m0),
                            )
                        st = stages[pr]
                        eng = nc.vector if (hl + pr) % 2 == 0 else nc.scalar
                        copy = eng.tensor_copy if eng is nc.vector else eng.copy
                        copy(st[:, hl, :], bank[:, :])
                for pr in range(PAIRS):
                    nc.scalar.dma_start(out_bt[pr, :, h0 : h0 + HC, :], stages[pr][:])

    nc.compile()
    return nc


def _get_module():
    if "nc" not in _cached:
        _cached["nc"] = _build_module()
    return _cached["nc"]


def _host_extract(bt):
    """Gather band diagonals into the full volume.

    bt: [16, 128, H, 160].  Row p holds G[w', w = BASE[k] + x] at col
    80*(k//4) + x where k = floor(w'/32) indexes the piece (w' = row for
    pieces 0-3 at cols 0:80, row for pieces 4-7 at cols 80:160).
    vol[d,h,w] = G[w-d, w] -> row (w-d) % 128, col from piece table.
    """
    d = np.arange(D)[:, None]
    w = np.arange(W)[None, :]
    wp = w - d  # [D, W] source w' (negative -> zero region)
    valid = wp >= 0
    wpc = np.clip(wp, 0, None)
    k = wpc // 32
    base = np.minimum(32 * k, W - PW)
    col = PW * (k // 4) + (w - base)
    row = wpc % 128
    assert (col[valid] >= PW * (k[valid] // 4)).all() and (
        col[valid] < PW * (k[valid] // 4) + PW
    ).all()

    vol = np.zeros((B * G, D, H, W), np.float32)
    for pair in range(B * G):
        t = bt[pair].transpose(1, 0, 2)  # [h, row, col]
        r = t[:, row, col]  # [H, D, W]
        r *= valid[None]
        vol[pair] = r.transpose(1, 0, 2)
    return vol.reshape(B, G, D, H, W)


def kernel(refimg_fea, targetimg_fea, num_groups, maxdisp):
    assert int(num_groups) == G and int(maxdisp) == D
    ref = np.ascontiguousarray(refimg_fea, dtype=np.float32)
    tgt = np.ascontiguousarray(targetimg_fea, dtype=np.float32)
    assert ref.shape == (B, C, H, W)

    rp = ref.reshape(B * G, CG, H, W)
    tp = tgt.reshape(B * G, CG, H, W)
    in_maps = [
        {"ref": rp[2 * k : 2 * k + 2], "tgt": tp[2 * k : 2 * k + 2]}
        for k in range(N_CORES)
    ]

    nc = _get_module()
    res = run_bass_kernel_spmd(nc, in_maps, core_ids=list(range(N_CORES)))

    return _host_extract(
        np.concatenate([r["out_bt"] for r in res.results], axis=0)
    )
